# revision 10
# baseline (speedup 1.0000x reference)
"""FBPINN (windowed mixture of per-subdomain MLPs) Trainium2 kernel.

Strategy: the cosine partition-of-unity window has compact support — a
point contributes to a subdomain's MLP only if it lies strictly inside
that subdomain's box.  With the 8x4 overlapped tiling each point lands in
~2.5 of the 32 boxes, so dense evaluation wastes >90% of the FLOPs.

  host:   route points -> per-expert gathered (padded) point lists
  device: 8 cores x 4 experts each; experts packed in pairs into the
          128-partition systolic array (block-diagonal weights); tiny
          MLP in feature-major layout, tanh+bias fused on the ACT engine
          reading PSUM directly.
  host:   scatter-add  w*(o+bo)  and  w  per point, final normalize.

Matmul operands use float32r (single-pass PE streaming, 4x the fp32
rate, ~11-bit effective multiplier).  Layer 0 stays accurate because x
and the folded W0 are hi/lo bf16-split on the host (the hi parts pass
through the reduced multiplier exactly); K grows 4->12 which is free
since PE cost scales with moving columns, not K.  Experts are pair-slot
balanced by point count so padding is minimal; all DMA traffic is
packed to minimize per-dma_start queue-issue overhead.
"""

import numpy as np

import concourse.bacc as bacc
import concourse.mybir as mybir
import concourse.tile as tile
from concourse.bass_utils import run_bass_kernel_spmd

# problem constants (hardcoded per contract)
N_PTS = 32768
S = 32
XDIM = 2
WIDTH = 64
TRANS = 0.1
TOL = 1e-8
N_CORES = 8
E_PER_CORE = S // N_CORES      # 4 experts per core
PAIRS = E_PER_CORE // 2        # 2 block-diag pairs (slots) per core

MM_CH = 512                    # fp32 moving-operand max per matmul
ACT_CH = 1024                  # ACT reads 2 psum banks per instruction
K0 = 12                        # L0 contraction rows after hi/lo splitting

# packed per-pair weight layout (128 partitions x WCOLS):
#   [0:128)   W1 block-diag   [128:256) W2 block-diag
#   [256:384) W0 (rows 0:K0)  [384:386) Wo columns
#   [386:389) b0 | b1 | b2
WCOLS = 389

_compiled_cache: dict[tuple, object] = {}


def _build_nc(slot_pads: tuple[int, ...]):
    fp32 = mybir.dt.float32
    fp32r = mybir.dt.float32r
    nc = bacc.Bacc("TRN2", target_bir_lowering=False, debug=False,
                   num_devices=N_CORES)

    wp_d = nc.dram_tensor("wpack", [PAIRS, 128, WCOLS], fp32r,
                          kind="ExternalInput")
    xt_d = [nc.dram_tensor(f"xt{p}", [K0, slot_pads[p]], fp32r,
                           kind="ExternalInput") for p in range(PAIRS)]
    oo_d = [nc.dram_tensor(f"oo{p}", [2, slot_pads[p]], fp32,
                           kind="ExternalOutput") for p in range(PAIRS)]

    tanh = mybir.ActivationFunctionType.Tanh

    with tile.TileContext(nc) as tc:
        with (
            tc.tile_pool(name="wpool", bufs=2) as wpool,
            tc.tile_pool(name="hpool", bufs=2) as hpool,
            tc.tile_pool(name="ps", bufs=1, space="PSUM") as ps,
            tc.tile_pool(name="pso", bufs=2, space="PSUM") as psop,
        ):
            for p in range(PAIRS):
                pad = slot_pads[p]
                wp = wpool.tile([128, WCOLS], fp32r, tag="wp")
                # one SWDGE transfer for every weight/bias of the pair
                nc.gpsimd.dma_start(wp[:], wp_d[p])
                w1 = wp[:, 0:128]
                w2 = wp[:, 128:256]
                w0 = wp[0:K0, 256:384]
                wo = wp[:, 384:386]
                b0 = wp[:, 386:387]
                b1 = wp[:, 387:388]
                b2 = wp[:, 388:389]

                xt = hpool.tile([K0, pad], fp32r, tag="xt")
                h0 = hpool.tile([128, pad], fp32r, tag="h0")
                h1 = hpool.tile([128, pad], fp32r, tag="h1")
                h2 = hpool.tile([128, pad], fp32r, tag="h2")
                o_sb = hpool.tile([2, pad], fp32, tag="o_sb")

                # ACT-chunk grid (tail chunk may be 512)
                acts = []
                a0 = 0
                while a0 < pad:
                    acts.append((a0, min(ACT_CH, pad - a0)))
                    a0 += ACT_CH

                for a0, alen in acts:
                    # chunked input DMA so compute starts after 1st chunk
                    nc.sync.dma_start(xt[:, a0:a0 + alen],
                                      xt_d[p][:, a0:a0 + alen])

                for a0, alen in acts:
                    ps0 = ps.tile([128, alen], fp32, tag="ps0")
                    for m in range(0, alen, MM_CH):
                        nc.tensor.matmul(ps0[:, m:m + MM_CH], w0,
                                         xt[:, a0 + m:a0 + m + MM_CH],
                                         start=True, stop=True)
                    nc.scalar.activation(h0[:, a0:a0 + alen], ps0[:],
                                         tanh, bias=b0)

                    ps1 = ps.tile([128, alen], fp32, tag="ps1")
                    for m in range(0, alen, MM_CH):
                        nc.tensor.matmul(ps1[:, m:m + MM_CH], w1,
                                         h0[:, a0 + m:a0 + m + MM_CH],
                                         start=True, stop=True)
                    nc.scalar.activation(h1[:, a0:a0 + alen], ps1[:],
                                         tanh, bias=b1)

                    ps2 = ps.tile([128, alen], fp32, tag="ps2")
                    for m in range(0, alen, MM_CH):
                        nc.tensor.matmul(ps2[:, m:m + MM_CH], w2,
                                         h1[:, a0 + m:a0 + m + MM_CH],
                                         start=True, stop=True)
                    nc.scalar.activation(h2[:, a0:a0 + alen], ps2[:],
                                         tanh, bias=b2)

                    for m in range(0, alen, MM_CH):
                        o = a0 + m
                        pso = psop.tile([2, MM_CH], fp32, tag="pso")
                        nc.tensor.matmul(pso[:], wo, h2[:, o:o + MM_CH],
                                         start=True, stop=True)
                        nc.vector.tensor_copy(o_sb[:, o:o + MM_CH], pso[:])
                    nc.sync.dma_start(oo_d[p][:, a0:a0 + alen],
                                      o_sb[:, a0:a0 + alen])
    nc.compile()
    return nc


def _get_nc(slot_pads):
    key = tuple(slot_pads)
    nc = _compiled_cache.get(key)
    if nc is None:
        nc = _build_nc(key)
        _compiled_cache[key] = nc
    return nc


def _assign_experts(counts):
    """Pair experts and assign to (core, slot) balancing point counts.

    Returns assign[core][slot] = (expert_a, expert_b) and slot_pads.
    Sort experts by count desc; adjacent pairing minimizes within-pair
    padding; the 8 largest pairs go to slot 0, the rest to slot 1, so
    each slot's cross-core pad (max over cores) stays tight.
    """
    order = sorted(range(S), key=lambda s: (-counts[s], s))
    pairs = [(order[2 * i], order[2 * i + 1]) for i in range(S // 2)]
    pairs.sort(key=lambda ab: -max(counts[ab[0]], counts[ab[1]]))
    assign = [[None] * PAIRS for _ in range(N_CORES)]
    slot_pads = []
    for p in range(PAIRS):
        chunk = pairs[p * N_CORES:(p + 1) * N_CORES]
        mx = max(max(counts[a], counts[b]) for a, b in chunk)
        slot_pads.append(int(max(MM_CH, -(-int(mx) // MM_CH) * MM_CH)))
        for c in range(N_CORES):
            assign[c][p] = chunk[c]
    return assign, tuple(slot_pads)


def kernel(x, xmins, xmaxs, W0, b0, W1, b1, W2, b2, Wo, bo):
    import ml_dtypes

    x = np.asarray(x)
    n_pts = x.shape[0]
    xmins64 = np.asarray(xmins, np.float64)
    xmaxs64 = np.asarray(xmaxs, np.float64)
    x64 = np.asarray(x, np.float64)

    # ---- host routing: strict-interior membership == window support ----
    inside = ((x[:, None, :] > xmins[None, :, :])
              & (x[:, None, :] < xmaxs[None, :, :])).all(-1)      # (N, S)
    idx = [np.nonzero(inside[:, s])[0] for s in range(S)]
    counts = np.array([len(i) for i in idx])
    assign, slot_pads = _assign_experts(counts)

    # ---- fold input normalization into layer-0 weights (float64) ----
    center = 0.5 * (xmins64 + xmaxs64)                            # (S, 2)
    scale = np.maximum(0.5 * (xmaxs64 - xmins64), 1e-9)
    W0f = np.asarray(W0, np.float64) / scale[:, None, :]          # (S, 64, 2)
    b0f = np.asarray(b0, np.float64) - (W0f * center[:, None, :]).sum(-1)

    def _split(v):
        hi = np.asarray(v, np.float32).astype(ml_dtypes.bfloat16) \
            .astype(np.float32)
        return hi, (np.asarray(v, np.float32) - hi)

    W1 = np.asarray(W1)
    W2 = np.asarray(W2)
    Wo = np.asarray(Wo)
    b1 = np.asarray(b1)
    b2 = np.asarray(b2)
    in_maps = []
    for core in range(N_CORES):
        m = {"wpack": np.zeros((PAIRS, 128, WCOLS), np.float32)}
        wpk = m["wpack"]
        for p in range(PAIRS):
            pad = slot_pads[p]
            xt = np.zeros((K0, pad), np.float32)
            for j, s in enumerate(assign[core][p]):
                lo, hi = 64 * j, 64 * (j + 1)
                pts = x[idx[s]]                                   # (P_s, 2)
                x_hi, x_lo = _split(pts.T)                        # (2, P_s)
                w_hi, w_lo = _split(W0f[s].T)                     # (2, 64)
                r0 = 6 * j
                n = pts.shape[0]
                # rows: [x_hi|W_hi, x_lo|W_hi, x_hi|W_lo]  (lo*lo ~ 2^-16)
                xt[r0 + 0:r0 + 2, :n] = x_hi
                xt[r0 + 2:r0 + 4, :n] = x_lo
                xt[r0 + 4:r0 + 6, :n] = x_hi
                wpk[p, lo:hi, 0 + lo:0 + hi] = W1[s].T
                wpk[p, lo:hi, 128 + lo:128 + hi] = W2[s].T
                wpk[p, r0 + 0:r0 + 2, 256 + lo:256 + hi] = w_hi
                wpk[p, r0 + 2:r0 + 4, 256 + lo:256 + hi] = w_hi
                wpk[p, r0 + 4:r0 + 6, 256 + lo:256 + hi] = w_lo
                wpk[p, lo:hi, 384 + j] = Wo[s, 0, :]
                wpk[p, lo:hi, 386] = b0f[s]
                wpk[p, lo:hi, 387] = b1[s]
                wpk[p, lo:hi, 388] = b2[s]
            m[f"xt{p}"] = xt
        in_maps.append(m)

    # ---- run on 8 cores ----
    global _last_in_maps
    _last_in_maps = in_maps
    nc = _get_nc(slot_pads)
    res = run_bass_kernel_spmd(nc, in_maps, core_ids=list(range(N_CORES)),
                               trace=False)

    # ---- window values + host scatter-add + normalize ----
    def window_vals(pts64, s):
        tu = np.clip((pts64 - xmins64[s]) / TRANS, 0.0, 1.0)
        td = np.clip((xmaxs64[s] - pts64) / TRANS, 0.0, 1.0)
        per = 0.25 * (1.0 - np.cos(np.pi * tu)) * (1.0 - np.cos(np.pi * td))
        return per.prod(-1)

    num = np.zeros(n_pts, np.float64)
    den = np.zeros(n_pts, np.float64)
    bo = np.asarray(bo, np.float64)
    for core in range(N_CORES):
        for p in range(PAIRS):
            oo = res.results[core][f"oo{p}"]                      # (2, pad)
            for j, s in enumerate(assign[core][p]):
                ii = idx[s]
                if len(ii) == 0:
                    continue
                w = window_vals(x64[ii], s)                       # (P_s,)
                num[ii] += w * (oo[j, :len(ii)].astype(np.float64) + bo[s, 0])
                den[ii] += w
    y = num / (den + TOL)
    return y.astype(np.float32).reshape(n_pts, 1)
